# revision 11
# baseline (speedup 1.0000x reference)
"""GPT attention block (B=2, S=2048, D=1024, H=16) on 8 TRN2 NeuronCores.

Sharding: tensor-parallel over heads. Core c computes heads {2c, 2c+1} for
both batches: its own slices of the q/k/v projections, the full S^2 score /
softmax matrices for those heads (written directly as the attn / attn_prob
outputs), attention@V, and a row-parallel partial of the fc output that the
host sums over cores.

Device-side layouts (per core):
  qhT, khT: [128(=2 heads x 64 head-dims), S]   ("head-transposed")
  vh:       [S, 128]  natural (s on partitions), stored [128, 16, 128]
  scores natural  [sq, sk]  -> attn / attn_prob outputs + row sums (free-dim)
  scores transposed [sk, sq] -> exp -> AV matmul (contraction over sk)
  avT/attT: [128(head dims), S] -> fc -> partial out [dout, S] (transposed)

Host passes q/k/v transposed to [B, D, S], weights pre-transposed/sliced and
1/sqrt(dh) folded into Wq/bq; host sums the 8 partial fc outputs at the end.
"""

import sys
import types

import ml_dtypes
import numpy as np

NPMM = ml_dtypes.bfloat16

import concourse.bass as bass
import concourse.mybir as mybir
import concourse.tile as tile
from concourse import bacc, bass_utils
from concourse.masks import make_identity

B, S, D, H = 2, 2048, 1024, 16
DH = D // H  # 64
NEG = -1e9
N_CORES = 8
HPC = H // N_CORES      # heads per core = 2
DPC = HPC * DH          # head-dims per core = 128
P = 128                 # partitions
SB = 512                # free-dim block
NSB = S // SB           # 4
NST = S // P            # 16
KO = D // P             # 8 contraction chunks for projections
F32 = mybir.dt.float32
F32R = mybir.dt.float32r
BF16 = mybir.dt.bfloat16
MMDT = BF16            # dtype for matmul operands
EXP = mybir.ActivationFunctionType.Exp
AX = mybir.AxisListType.X
MUL = mybir.AluOpType.mult
ADD = mybir.AluOpType.add

# matmul dtype switches (float32r = full-rate fp32-storage matmul)
R_PROJ = True
R_SCORE = True
R_AV = True
R_FC = True


def _mm(nc, out, lhsT, rhs, relaxed, **kw):
    nc.tensor.matmul(out, lhsT, rhs, **kw)


def build_nc():
    nc = bacc.Bacc("TRN2", target_bir_lowering=False, debug=False,
                   enable_asserts=False, num_devices=N_CORES)

    qT = nc.dram_tensor("qT", [B, D, S], MMDT, kind="ExternalInput").ap()
    kT = nc.dram_tensor("kT", [B, D, S], MMDT, kind="ExternalInput").ap()
    vT = nc.dram_tensor("vT", [B, D, S], MMDT, kind="ExternalInput").ap()
    wqT = nc.dram_tensor("wqT", [D, DPC], MMDT, kind="ExternalInput").ap()
    wkT = nc.dram_tensor("wkT", [D, DPC], MMDT, kind="ExternalInput").ap()
    wvT = nc.dram_tensor("wvT", [D, DPC], MMDT, kind="ExternalInput").ap()
    bq = nc.dram_tensor("bq", [DPC, 1], F32, kind="ExternalInput").ap()
    bk = nc.dram_tensor("bk", [DPC, 1], F32, kind="ExternalInput").ap()
    bv = nc.dram_tensor("bv", [DPC, 1], F32, kind="ExternalInput").ap()
    wfcT = nc.dram_tensor("wfcT", [DPC, D], MMDT, kind="ExternalInput").ap()
    madd_bc = nc.dram_tensor("madd_bc", [B, P, S], F32, kind="ExternalInput").ap()
    madd_col = nc.dram_tensor("madd_col", [B, P, NST], F32, kind="ExternalInput").ap()

    attn_o = nc.dram_tensor("attn_o", [B * HPC, S, S], F32, kind="ExternalOutput").ap()
    prob_o = nc.dram_tensor("prob_o", [B * HPC, S, S], F32, kind="ExternalOutput").ap()
    fcp_o = nc.dram_tensor("fcp_o", [B, D, S], F32, kind="ExternalOutput").ap()

    from contextlib import ExitStack
    with tile.TileContext(nc) as tc, ExitStack() as ctx:
        consts = ctx.enter_context(tc.tile_pool(name="consts", bufs=1))
        projin = ctx.enter_context(tc.tile_pool(name="projin", bufs=3))
        resid = ctx.enter_context(tc.tile_pool(name="resid", bufs=2))
        strips = ctx.enter_context(tc.tile_pool(name="strips", bufs=2))
        small = ctx.enter_context(tc.tile_pool(name="small", bufs=3))
        ep_pool = ctx.enter_context(tc.tile_pool(name="ep", bufs=3))
        bc_pool = ctx.enter_context(tc.tile_pool(name="bc", bufs=2))
        dram = ctx.enter_context(tc.tile_pool(name="dram", bufs=2, space="DRAM"))
        # psA: [128,1024] slots (2 banks) x3 = 6 banks; psB: 1-bank slots x2
        psA = ctx.enter_context(tc.tile_pool(name="psA", bufs=3, space="PSUM"))
        psB = ctx.enter_context(tc.tile_pool(name="psB", bufs=2, space="PSUM"))

        HB = 2 * SB  # 1024 wide psum strips

        # ---- constants in SBUF ----
        w_sb = {}
        for name, drm in (("q", wqT), ("k", wkT), ("v", wvT)):
            t = consts.tile([P, KO, DPC], MMDT, tag=f"w{name}")
            nc.sync.dma_start(t[:], drm.rearrange("(ko p) m -> p ko m", p=P))
            w_sb[name] = t
        b_sb = {}
        for name, drm in (("q", bq), ("k", bk), ("v", bv)):
            t = consts.tile([DPC, 1], F32, tag=f"b{name}")
            nc.sync.dma_start(t[:], drm)
            b_sb[name] = t
        wfc_sb = consts.tile([P, D], MMDT, tag="wfc")
        nc.sync.dma_start(wfc_sb[:], wfcT)
        madd_sb = consts.tile([P, B, S], F32, tag="madd")
        for b in range(B):
            nc.sync.dma_start(madd_sb[:, b, :], madd_bc[b])
        maddc_sb = consts.tile([P, B, NST], F32, tag="maddc")
        for b in range(B):
            nc.sync.dma_start(maddc_sb[:, b, :], madd_col[b])
        ident = consts.tile([P, P], MMDT, tag="ident")
        make_identity(nc, ident[:])

        for b in range(B):
            # ======== projections ========
            proj_out = {}
            for name in ("q", "k", "v"):
                proj_out[name] = resid.tile([P, S], MMDT, tag=f"{name}hT",
                                            name=f"{name}hT")
            for name, drm in (("q", qT), ("k", kT), ("v", vT)):
                psums = [psA.tile([P, HB], F32, tag="psA", name=f"pp{j}")
                         for j in range(2)]
                for ko in range(KO):
                    chunk = projin.tile([P, S], MMDT, tag="projin")
                    nc.sync.dma_start(chunk[:], drm[b, ko * P:(ko + 1) * P, :])
                    for j in range(NSB):
                        _mm(nc, psums[j // 2][:, (j % 2) * SB:(j % 2 + 1) * SB],
                            w_sb[name][:, ko, :],
                            chunk[:, j * SB:(j + 1) * SB], R_PROJ,
                            start=(ko == 0), stop=(ko == KO - 1))
                for j in range(2):
                    nc.scalar.add(proj_out[name][:, j * HB:(j + 1) * HB],
                                  psums[j][:], b_sb[name][:])
            qhT, khT = proj_out["q"], proj_out["k"]
            # vh natural [s, dpc] via PE transpose of vhT
            vh = resid.tile([P, NST, P], MMDT, tag="vh")
            for st in range(NST):
                pt = psB.tile([P, P], MMDT, tag="psB", name="pt")
                nc.tensor.transpose(pt[:], proj_out["v"][:, st * P:(st + 1) * P],
                                    ident[:])
                nc.vector.tensor_copy(vh[:, st, :], pt[:])

            attT = resid.tile([P, S], MMDT, tag="attT")
            avu = resid.tile([P, S], F32, tag="avu")

            HPs = [slice(DH * hl, DH * (hl + 1)) for hl in range(HPC)]
            recips = [small.tile([P, NST], F32, tag=f"recip{hl}",
                                 name=f"recip{hl}") for hl in range(HPC)]
            rec_bcs = [None, None]

            def stage1(st):
                sq = slice(st * P, (st + 1) * P)
                a_s, e_s, accs = [], [], []
                for hl in range(HPC):
                    a_s.append(strips.tile([P, S], F32, tag="attn_strip",
                                           name=f"as{hl}"))
                    e_s.append(ep_pool.tile([P, S], F32, tag="ep_strip",
                                            name=f"es{hl}"))
                    accs.append(small.tile([P, 2], F32, tag=f"acc{hl}",
                                           name=f"acc{hl}"))
                for half in range(2):
                    hs = slice(half * HB, (half + 1) * HB)
                    ps = [psA.tile([P, HB], F32, tag="psA", name=f"s1p{hl}")
                          for hl in range(HPC)]
                    for kb in range(2):
                        sk = slice(half * HB + kb * SB, half * HB + (kb + 1) * SB)
                        for hl in range(HPC):  # adjacent emission -> row-packed
                            _mm(nc, ps[hl][:, kb * SB:(kb + 1) * SB],
                                qhT[HPs[hl], sq], khT[HPs[hl], sk], R_SCORE,
                                start=True, stop=True)
                    for hl in range(HPC):
                        nc.vector.tensor_tensor(a_s[hl][:, hs], ps[hl][:],
                                                madd_sb[:, b, hs], ADD)
                        nc.scalar.activation(e_s[hl][:, hs], ps[hl][:], EXP,
                                             accum_out=accs[hl][:, half:half + 1])
                for hl in range(HPC):
                    pair = b * HPC + hl
                    nc.sync.dma_start(attn_o[pair, sq, :], a_s[hl][:])
                    sums = small.tile([P, 1], F32, tag="sums", name="sums")
                    nc.vector.reduce_sum(sums[:], accs[hl][:], axis=AX)
                    nc.vector.reciprocal(recips[hl][:, st:st + 1], sums[:])
                    prob_strip = strips.tile([P, S], F32, tag="prob_strip",
                                             name="prob_strip")
                    if (st + hl) % 2 == 0:
                        nc.vector.tensor_tensor(
                            prob_strip[:], e_s[hl][:],
                            recips[hl][:, st:st + 1].to_broadcast([P, S]), MUL)
                    else:
                        nc.scalar.mul(prob_strip[:], e_s[hl][:],
                                      recips[hl][:, st:st + 1])
                    nc.sync.dma_start(prob_o[pair, sq, :], prob_strip[:])

            def stage2(qb):
                sq = slice(qb * SB, (qb + 1) * SB)
                pav = psB.tile([P, SB], F32, tag="psB", name="pav")
                for st in range(NST):
                    skt = slice(st * P, (st + 1) * P)
                    psn = psA.tile([P, HB], F32, tag="psA", name="psn")
                    for hl in range(HPC):  # row-packed pair
                        _mm(nc, psn[:, hl * SB:(hl + 1) * SB],
                            khT[HPs[hl], skt], qhT[HPs[hl], sq], R_SCORE,
                            start=True, stop=True)
                    ept = small.tile([P, HB], MMDT, tag="ept", name="ept")
                    nc.scalar.activation(ept[:], psn[:], EXP,
                                         bias=maddc_sb[:, b, st:st + 1])
                    for hl in range(HPC):  # col-packed pair, accumulated
                        _mm(nc, pav[hl * DH:(hl + 1) * DH, :],
                            vh[:, st, HPs[hl]], ept[:, hl * SB:(hl + 1) * SB],
                            R_AV, start=(st == 0), stop=(st == NST - 1),
                            tile_position=(0, hl * DH), skip_group_check=True)
                nc.vector.tensor_copy(avu[:, sq], pav[:])

            for st in range(NST):
                stage1(st)
                if st % (NST // NSB) == (NST // NSB) - 1:
                    stage2(st // (NST // NSB))

            for hl in range(HPC):
                # recip roundtrip: [128,16] (p=sq%128) -> linear [S] -> bcast
                rec_lin = dram.tile([S], F32, tag="rec_lin")
                nc.sync.dma_start(
                    rec_lin.rearrange("(so p) -> p so", p=P), recips[hl][:])
                rec_row = bc_pool.tile([1, S], F32, tag="rec_row", name="rr")
                nc.sync.dma_start(rec_row[:], rec_lin[None, :])
                rec_bc = bc_pool.tile([P, S], F32, tag="rec_bc", name="rb")
                nc.gpsimd.partition_broadcast(rec_bc[:], rec_row[:])
                rec_bcs[hl] = rec_bc
            for qb in range(NSB):
                sq = slice(qb * SB, (qb + 1) * SB)
                for hl in range(HPC):
                    nc.vector.tensor_tensor(attT[HPs[hl], sq], avu[HPs[hl], sq],
                                            rec_bcs[hl][HPs[hl], sq], MUL)

            # ======== fc (row-parallel partial, output transposed) ========
            for ot in range(KO):
                do = slice(ot * P, (ot + 1) * P)
                for qb in range(NSB):
                    sq = slice(qb * SB, (qb + 1) * SB)
                    pfc = psA.tile([P, SB], F32, tag="psA", name="pfc")
                    _mm(nc, pfc[:], wfc_sb[:, do], attT[:, sq], R_FC,
                        start=True, stop=True)
                    ostrip = small.tile([P, SB], F32, tag="ostrip")
                    nc.scalar.copy(ostrip[:], pfc[:])
                    nc.sync.dma_start(fcp_o[b, do, sq], ostrip[:])

    nc.compile()
    return nc


_NC_CACHE = None


def _install_hook():
    try:
        import antenv.axon_hooks  # noqa: F401
        return
    except ImportError:
        pass
    try:
        from trn_agent_boot.trn_boot import _ntff_profile_via_ctypes
        hook = _ntff_profile_via_ctypes('/opt/axon/libaxon_pjrt.so')
    except Exception:
        hook = None
    m = types.ModuleType('antenv.axon_hooks')
    m.get_axon_ntff_profile_hook = lambda: hook
    m.set_axon_ntff_profile_hook = lambda h: None
    sys.modules['antenv.axon_hooks'] = m


def kernel(q, k, v, mask, Wq, bq, Wk, bk, Wv, bv, Wfc, bfc, _trace=False):
    _install_hook()
    global _NC_CACHE
    if _NC_CACHE is None:
        _NC_CACHE = build_nc()
    nc = _NC_CACHE

    q = np.asarray(q, np.float32)
    k = np.asarray(k, np.float32)
    v = np.asarray(v, np.float32)
    mask = np.asarray(mask)
    temp = np.float32(DH) ** 0.5
    qT = np.ascontiguousarray(q.transpose(0, 2, 1)).astype(NPMM)
    kT = np.ascontiguousarray(k.transpose(0, 2, 1)).astype(NPMM)
    vT = np.ascontiguousarray(v.transpose(0, 2, 1)).astype(NPMM)
    madd = ((1.0 - mask.astype(np.float32)) * NEG).astype(np.float32)  # [B,S]
    madd_bc = np.ascontiguousarray(
        np.broadcast_to(madd[:, None, :], (B, P, S)), np.float32)
    madd_col = np.ascontiguousarray(
        madd.reshape(B, NST, P).transpose(0, 2, 1), np.float32)

    Wq = np.asarray(Wq, np.float32)
    Wk = np.asarray(Wk, np.float32)
    Wv = np.asarray(Wv, np.float32)
    Wfc = np.asarray(Wfc, np.float32)
    bqf = np.asarray(bq, np.float32) / temp
    in_maps = []
    for c in range(N_CORES):
        cols = slice(c * DPC, (c + 1) * DPC)
        in_maps.append({
            "qT": qT, "kT": kT, "vT": vT,
            "wqT": np.ascontiguousarray(Wq[cols, :].T / temp).astype(NPMM),
            "wkT": np.ascontiguousarray(Wk[cols, :].T).astype(NPMM),
            "wvT": np.ascontiguousarray(Wv[cols, :].T).astype(NPMM),
            "bq": np.ascontiguousarray(bqf[cols])[:, None],
            "bk": np.ascontiguousarray(np.asarray(bk, np.float32)[cols])[:, None],
            "bv": np.ascontiguousarray(np.asarray(bv, np.float32)[cols])[:, None],
            "wfcT": np.ascontiguousarray(Wfc[:, cols].T).astype(NPMM),
            "madd_bc": madd_bc, "madd_col": madd_col,
        })

    res = bass_utils.run_bass_kernel_spmd(
        nc, in_maps, core_ids=list(range(N_CORES)), trace=_trace)
    kernel.last_result = res

    out = np.zeros((B, S, D), np.float32)
    attn = np.empty((B, H, S, S), np.float32)
    prob = np.empty((B, H, S, S), np.float32)
    for c in range(N_CORES):
        r = res.results[c]
        for b in range(B):
            for hl in range(HPC):
                h = HPC * c + hl
                attn[b, h] = r["attn_o"][b * HPC + hl]
                prob[b, h] = r["prob_o"][b * HPC + hl]
            out[b] += r["fcp_o"][b].T
    out += np.asarray(bfc, np.float32)
    return out, prob, attn


# revision 12
# speedup vs baseline: 1.0544x; 1.0544x over previous
"""GPT attention block (B=2, S=2048, D=1024, H=16) on 8 TRN2 NeuronCores.

Sharding: tensor-parallel over heads. Core c computes heads {2c, 2c+1} for
both batches: its own slices of the q/k/v projections, the full S^2 score /
softmax matrices for those heads (written directly as the attn / attn_prob
outputs), attention@V, and a row-parallel partial of the fc output that the
host sums over cores.

Device-side layouts (per core):
  qhT, khT: [128(=2 heads x 64 head-dims), S]   ("head-transposed")
  vh:       [S, 128]  natural (s on partitions), stored [128, 16, 128]
  scores natural  [sq, sk]  -> attn / attn_prob outputs + row sums (free-dim)
  scores transposed [sk, sq] -> exp -> AV matmul (contraction over sk)
  avT/attT: [128(head dims), S] -> fc -> partial out [dout, S] (transposed)

Host passes q/k/v transposed to [B, D, S], weights pre-transposed/sliced and
1/sqrt(dh) folded into Wq/bq; host sums the 8 partial fc outputs at the end.
"""

import sys
import types

import ml_dtypes
import numpy as np

NPMM = ml_dtypes.bfloat16

import concourse.bass as bass
import concourse.mybir as mybir
import concourse.tile as tile
from concourse import bacc, bass_utils
from concourse.masks import make_identity

B, S, D, H = 2, 2048, 1024, 16
DH = D // H  # 64
NEG = -1e9
N_CORES = 8
HPC = H // N_CORES      # heads per core = 2
DPC = HPC * DH          # head-dims per core = 128
P = 128                 # partitions
SB = 512                # free-dim block
NSB = S // SB           # 4
NST = S // P            # 16
KO = D // P             # 8 contraction chunks for projections
F32 = mybir.dt.float32
F32R = mybir.dt.float32r
BF16 = mybir.dt.bfloat16
MMDT = BF16            # dtype for matmul operands
EXP = mybir.ActivationFunctionType.Exp
AX = mybir.AxisListType.X
MUL = mybir.AluOpType.mult
ADD = mybir.AluOpType.add

# matmul dtype switches (float32r = full-rate fp32-storage matmul)
R_PROJ = True
R_SCORE = True
R_AV = True
R_FC = True


def _mm(nc, out, lhsT, rhs, relaxed, **kw):
    nc.tensor.matmul(out, lhsT, rhs, **kw)


def build_nc():
    nc = bacc.Bacc("TRN2", target_bir_lowering=False, debug=False,
                   enable_asserts=False, num_devices=N_CORES)

    qT = nc.dram_tensor("qT", [B, D, S], MMDT, kind="ExternalInput").ap()
    kT = nc.dram_tensor("kT", [B, D, S], MMDT, kind="ExternalInput").ap()
    vT = nc.dram_tensor("vT", [B, D, S], MMDT, kind="ExternalInput").ap()
    wqT = nc.dram_tensor("wqT", [D, DPC], MMDT, kind="ExternalInput").ap()
    wkT = nc.dram_tensor("wkT", [D, DPC], MMDT, kind="ExternalInput").ap()
    wvT = nc.dram_tensor("wvT", [D, DPC], MMDT, kind="ExternalInput").ap()
    bq = nc.dram_tensor("bq", [DPC, 1], F32, kind="ExternalInput").ap()
    bk = nc.dram_tensor("bk", [DPC, 1], F32, kind="ExternalInput").ap()
    bv = nc.dram_tensor("bv", [DPC, 1], F32, kind="ExternalInput").ap()
    wfcT = nc.dram_tensor("wfcT", [DPC, D], MMDT, kind="ExternalInput").ap()
    madd_bc = nc.dram_tensor("madd_bc", [B, P, S], F32, kind="ExternalInput").ap()
    madd_col = nc.dram_tensor("madd_col", [B, P, NST], F32, kind="ExternalInput").ap()

    attn_o = nc.dram_tensor("attn_o", [B * HPC, S, S], F32, kind="ExternalOutput").ap()
    prob_o = nc.dram_tensor("prob_o", [B * HPC, S, S], F32, kind="ExternalOutput").ap()
    fcp_o = nc.dram_tensor("fcp_o", [B, D, S], F32, kind="ExternalOutput").ap()

    from contextlib import ExitStack
    with tile.TileContext(nc) as tc, ExitStack() as ctx:
        consts = ctx.enter_context(tc.tile_pool(name="consts", bufs=1))
        projin = ctx.enter_context(tc.tile_pool(name="projin", bufs=3))
        resid = ctx.enter_context(tc.tile_pool(name="resid", bufs=2))
        strips = ctx.enter_context(tc.tile_pool(name="strips", bufs=2))
        small = ctx.enter_context(tc.tile_pool(name="small", bufs=3))
        ep_pool = ctx.enter_context(tc.tile_pool(name="ep", bufs=3))
        bc_pool = ctx.enter_context(tc.tile_pool(name="bc", bufs=2))
        dram = ctx.enter_context(tc.tile_pool(name="dram", bufs=2, space="DRAM"))
        # psA: [128,1024] slots (2 banks) x3 = 6 banks; psB: 1-bank slots x2
        psA = ctx.enter_context(tc.tile_pool(name="psA", bufs=3, space="PSUM"))
        psB = ctx.enter_context(tc.tile_pool(name="psB", bufs=2, space="PSUM"))

        HB = 2 * SB  # 1024 wide psum strips

        # ---- constants in SBUF ----
        w_sb = {}
        for name, drm in (("q", wqT), ("k", wkT), ("v", wvT)):
            t = consts.tile([P, KO, DPC], MMDT, tag=f"w{name}")
            nc.sync.dma_start(t[:], drm.rearrange("(ko p) m -> p ko m", p=P))
            w_sb[name] = t
        b_sb = {}
        for name, drm in (("q", bq), ("k", bk), ("v", bv)):
            t = consts.tile([DPC, 1], F32, tag=f"b{name}")
            nc.sync.dma_start(t[:], drm)
            b_sb[name] = t
        wfc_sb = consts.tile([P, D], MMDT, tag="wfc")
        nc.sync.dma_start(wfc_sb[:], wfcT)
        madd_sb = consts.tile([P, B, S], F32, tag="madd")
        for b in range(B):
            nc.sync.dma_start(madd_sb[:, b, :], madd_bc[b])
        maddc_sb = consts.tile([P, B, NST], F32, tag="maddc")
        for b in range(B):
            nc.sync.dma_start(maddc_sb[:, b, :], madd_col[b])
        ident = consts.tile([P, P], MMDT, tag="ident")
        make_identity(nc, ident[:])

        # ======== projections for BOTH batches up front ========
        PJ = {}
        for b in range(B):
            proj_out = {}
            for name in ("q", "k", "v"):
                proj_out[name] = resid.tile([P, S], MMDT, tag=f"{name}hT",
                                            name=f"{name}hT{b}")
            for name, drm in (("q", qT), ("k", kT), ("v", vT)):
                psums = [psA.tile([P, HB], F32, tag="psA", name=f"pp{j}")
                         for j in range(2)]
                for ko in range(KO):
                    chunk = projin.tile([P, S], MMDT, tag="projin")
                    nc.sync.dma_start(chunk[:], drm[b, ko * P:(ko + 1) * P, :])
                    for j in range(NSB):
                        _mm(nc, psums[j // 2][:, (j % 2) * SB:(j % 2 + 1) * SB],
                            w_sb[name][:, ko, :],
                            chunk[:, j * SB:(j + 1) * SB], R_PROJ,
                            start=(ko == 0), stop=(ko == KO - 1))
                for j in range(2):
                    nc.scalar.add(proj_out[name][:, j * HB:(j + 1) * HB],
                                  psums[j][:], b_sb[name][:])
            vh = resid.tile([P, NST, P], MMDT, tag="vh", name=f"vh{b}")
            for st in range(NST):
                pt = psB.tile([P, P], MMDT, tag="psB", name="pt")
                nc.tensor.transpose(pt[:], proj_out["v"][:, st * P:(st + 1) * P],
                                    ident[:])
                nc.vector.tensor_copy(vh[:, st, :], pt[:])
            PJ[b] = (proj_out["q"], proj_out["k"], vh)

        # ======== attention + fc per batch ========
        for b in range(B):
            qhT, khT, vh = PJ[b]
            attT = resid.tile([P, S], MMDT, tag="attT", name=f"attT{b}")
            avu = resid.tile([P, S], F32, tag="avu", name=f"avu{b}")

            HPs = [slice(DH * hl, DH * (hl + 1)) for hl in range(HPC)]
            recips = [small.tile([P, NST], F32, tag=f"recip{hl}",
                                 name=f"recip{hl}") for hl in range(HPC)]
            rec_bcs = [None, None]

            def stage1(st):
                sq = slice(st * P, (st + 1) * P)
                a_s, e_s, accs = [], [], []
                for hl in range(HPC):
                    a_s.append(strips.tile([P, S], F32, tag="attn_strip",
                                           name=f"as{hl}"))
                    e_s.append(ep_pool.tile([P, S], F32, tag="ep_strip",
                                            name=f"es{hl}"))
                    accs.append(small.tile([P, 2], F32, tag=f"acc{hl}",
                                           name=f"acc{hl}"))
                for half in range(2):
                    hs = slice(half * HB, (half + 1) * HB)
                    ps = [psA.tile([P, HB], F32, tag="psA", name=f"s1p{hl}")
                          for hl in range(HPC)]
                    for kb in range(2):
                        sk = slice(half * HB + kb * SB, half * HB + (kb + 1) * SB)
                        for hl in range(HPC):  # adjacent emission -> row-packed
                            _mm(nc, ps[hl][:, kb * SB:(kb + 1) * SB],
                                qhT[HPs[hl], sq], khT[HPs[hl], sk], R_SCORE,
                                start=True, stop=True)
                    for hl in range(HPC):
                        nc.vector.tensor_tensor(a_s[hl][:, hs], ps[hl][:],
                                                madd_sb[:, b, hs], ADD)
                        nc.scalar.activation(e_s[hl][:, hs], ps[hl][:], EXP,
                                             accum_out=accs[hl][:, half:half + 1])
                for hl in range(HPC):
                    pair = b * HPC + hl
                    nc.sync.dma_start(attn_o[pair, sq, :], a_s[hl][:])
                    sums = small.tile([P, 1], F32, tag="sums", name="sums")
                    nc.vector.reduce_sum(sums[:], accs[hl][:], axis=AX)
                    nc.vector.reciprocal(recips[hl][:, st:st + 1], sums[:])
                    prob_strip = strips.tile([P, S], F32, tag="prob_strip",
                                             name="prob_strip")
                    nc.vector.tensor_scalar_mul(prob_strip[:], e_s[hl][:],
                                                recips[hl][:, st:st + 1])
                    nc.sync.dma_start(prob_o[pair, sq, :], prob_strip[:])

            def stage2(qb):
                sq = slice(qb * SB, (qb + 1) * SB)
                pav = psB.tile([P, SB], F32, tag="psB", name="pav")
                for st in range(NST):
                    skt = slice(st * P, (st + 1) * P)
                    psn = psA.tile([P, HB], F32, tag="psA", name="psn")
                    for hl in range(HPC):  # row-packed pair
                        _mm(nc, psn[:, hl * SB:(hl + 1) * SB],
                            khT[HPs[hl], skt], qhT[HPs[hl], sq], R_SCORE,
                            start=True, stop=True)
                    ept = small.tile([P, HB], MMDT, tag="ept", name="ept")
                    nc.scalar.activation(ept[:], psn[:], EXP,
                                         bias=maddc_sb[:, b, st:st + 1])
                    for hl in range(HPC):  # col-packed pair, accumulated
                        _mm(nc, pav[hl * DH:(hl + 1) * DH, :],
                            vh[:, st, HPs[hl]], ept[:, hl * SB:(hl + 1) * SB],
                            R_AV, start=(st == 0), stop=(st == NST - 1),
                            tile_position=(0, hl * DH), skip_group_check=True)
                nc.vector.tensor_copy(avu[:, sq], pav[:])

            for st in range(NST):
                stage1(st)
                if st % (NST // NSB) == (NST // NSB) - 1:
                    stage2(st // (NST // NSB))

            for hl in range(HPC):
                # recip roundtrip: [128,16] (p=sq%128) -> linear [S] -> bcast
                rec_lin = dram.tile([S], F32, tag="rec_lin")
                nc.sync.dma_start(
                    rec_lin.rearrange("(so p) -> p so", p=P), recips[hl][:])
                rec_row = bc_pool.tile([1, S], F32, tag="rec_row", name="rr")
                nc.sync.dma_start(rec_row[:], rec_lin[None, :])
                rec_bc = bc_pool.tile([P, S], F32, tag="rec_bc", name="rb")
                nc.gpsimd.partition_broadcast(rec_bc[:], rec_row[:])
                rec_bcs[hl] = rec_bc
            for qb in range(NSB):
                sq = slice(qb * SB, (qb + 1) * SB)
                for hl in range(HPC):
                    nc.vector.tensor_tensor(attT[HPs[hl], sq], avu[HPs[hl], sq],
                                            rec_bcs[hl][HPs[hl], sq], MUL)
                # fc for this sq block (row-parallel partial, transposed out)
                for ot in range(KO):
                    do = slice(ot * P, (ot + 1) * P)
                    pfc = psA.tile([P, SB], F32, tag="psA", name="pfc")
                    _mm(nc, pfc[:], wfc_sb[:, do], attT[:, sq], R_FC,
                        start=True, stop=True)
                    ostrip = small.tile([P, SB], F32, tag="ostrip")
                    nc.vector.tensor_copy(ostrip[:], pfc[:])
                    nc.sync.dma_start(fcp_o[b, do, sq], ostrip[:])

    nc.compile()
    return nc


_NC_CACHE = None


def _install_hook():
    try:
        import antenv.axon_hooks  # noqa: F401
        return
    except ImportError:
        pass
    try:
        from trn_agent_boot.trn_boot import _ntff_profile_via_ctypes
        hook = _ntff_profile_via_ctypes('/opt/axon/libaxon_pjrt.so')
    except Exception:
        hook = None
    m = types.ModuleType('antenv.axon_hooks')
    m.get_axon_ntff_profile_hook = lambda: hook
    m.set_axon_ntff_profile_hook = lambda h: None
    sys.modules['antenv.axon_hooks'] = m


def kernel(q, k, v, mask, Wq, bq, Wk, bk, Wv, bv, Wfc, bfc, _trace=False):
    _install_hook()
    global _NC_CACHE
    if _NC_CACHE is None:
        _NC_CACHE = build_nc()
    nc = _NC_CACHE

    q = np.asarray(q, np.float32)
    k = np.asarray(k, np.float32)
    v = np.asarray(v, np.float32)
    mask = np.asarray(mask)
    temp = np.float32(DH) ** 0.5
    qT = np.ascontiguousarray(q.transpose(0, 2, 1)).astype(NPMM)
    kT = np.ascontiguousarray(k.transpose(0, 2, 1)).astype(NPMM)
    vT = np.ascontiguousarray(v.transpose(0, 2, 1)).astype(NPMM)
    madd = ((1.0 - mask.astype(np.float32)) * NEG).astype(np.float32)  # [B,S]
    madd_bc = np.ascontiguousarray(
        np.broadcast_to(madd[:, None, :], (B, P, S)), np.float32)
    madd_col = np.ascontiguousarray(
        madd.reshape(B, NST, P).transpose(0, 2, 1), np.float32)

    Wq = np.asarray(Wq, np.float32)
    Wk = np.asarray(Wk, np.float32)
    Wv = np.asarray(Wv, np.float32)
    Wfc = np.asarray(Wfc, np.float32)
    bqf = np.asarray(bq, np.float32) / temp
    in_maps = []
    for c in range(N_CORES):
        cols = slice(c * DPC, (c + 1) * DPC)
        in_maps.append({
            "qT": qT, "kT": kT, "vT": vT,
            "wqT": np.ascontiguousarray(Wq[cols, :].T / temp).astype(NPMM),
            "wkT": np.ascontiguousarray(Wk[cols, :].T).astype(NPMM),
            "wvT": np.ascontiguousarray(Wv[cols, :].T).astype(NPMM),
            "bq": np.ascontiguousarray(bqf[cols])[:, None],
            "bk": np.ascontiguousarray(np.asarray(bk, np.float32)[cols])[:, None],
            "bv": np.ascontiguousarray(np.asarray(bv, np.float32)[cols])[:, None],
            "wfcT": np.ascontiguousarray(Wfc[:, cols].T).astype(NPMM),
            "madd_bc": madd_bc, "madd_col": madd_col,
        })

    res = bass_utils.run_bass_kernel_spmd(
        nc, in_maps, core_ids=list(range(N_CORES)), trace=_trace)
    kernel.last_result = res

    out = np.zeros((B, S, D), np.float32)
    attn = np.empty((B, H, S, S), np.float32)
    prob = np.empty((B, H, S, S), np.float32)
    for c in range(N_CORES):
        r = res.results[c]
        for b in range(B):
            for hl in range(HPC):
                h = HPC * c + hl
                attn[b, h] = r["attn_o"][b * HPC + hl]
                prob[b, h] = r["prob_o"][b * HPC + hl]
            out[b] += r["fcp_o"][b].T
    out += np.asarray(bfc, np.float32)
    return out, prob, attn
